# revision 16
# baseline (speedup 1.0000x reference)
"""DyConv (DyHead-style) Trainium2 kernel.

Sharding: data-parallel over batch B=8 -> 8 NeuronCores, one sample per core.
All parameters replicated. Per-core layout: channels on partitions
(C=256 = 2 blocks of 128), spatial on the free dimension.

Compute mapping per core:
  - depthwise 3x3 conv: 9 PSUM-accumulating diagonal matmuls (float32r)
  - SiLU: ScalarE activation (PSUM->SBUF), accum_out provides per-channel
    sums for the attention / DyReLU means for free
  - pointwise 1x1 conv: bf16 matmuls (K=256 via 2 accumulating matmuls)
  - bilinear upsample (align_corners): pair-structured AP views + interp maps
  - attention scalars + DyReLU FC chain: tiny matmuls + fused DVE ops
  - weighted branch max / 3x3 maxpool / DyReLU: fused vector-engine passes
"""

import numpy as np
import ml_dtypes

import concourse.bass as bass
import concourse.tile as tile
import concourse.mybir as mybir
from concourse.bass_utils import run_bass_kernel_spmd

F32 = mybir.dt.float32
F32R = mybir.dt.float32r
BF16 = mybir.dt.bfloat16
AX = mybir.AxisListType
ALU = mybir.AluOpType
ACT = mybir.ActivationFunctionType

C = 256
N_CORES = 8
NEG = -1e30

TAPS = [(dy, dx) for dy in range(3) for dx in range(3)]

# distinct conv applications: (key, conv_idx, src, stride, Hout)
W_ITEMS = {
    "c1x1": (1, "x1", 1, 32),
    "c1x0": (1, "x0", 1, 64),
    "c5x0": (5, "x0", 2, 32),
    "c4x1": (4, "x1", 1, 32),
    "c4x2": (4, "x2", 1, 16),
    "c7x2": (7, "x2", 1, 16),
    "c8x1": (8, "x1", 2, 16),
}
CONVS = [1, 4, 5, 7, 8]
CONV_SLOT = {c: i for i, c in enumerate(CONVS)}
SRC_H = {"x0": 64, "x1": 32, "x2": 16}

# levels: (H, [(work_key, upsample?)...])
LEVELS = [
    (64, [("c1x0", False), ("c1x1", True)]),
    (32, [("c4x1", False), ("c5x0", False), ("c4x2", True)]),
    (16, [("c7x2", False), ("c8x1", False)]),
]


def _split_multiwaits(nc):
    """This walrus build accepts only ONE sync-wait per instruction; Tile can
    attach several. Split extras into wait-only NoOps on the same engine,
    placed immediately before the instruction (identical semantics)."""
    n = 0
    for f in nc.m.functions:
        for bb in f.blocks:
            il = bb.instructions
            out = []
            for inst in il:
                si = inst.sync_info
                if si is not None and si.on_wait is not None and len(si.on_wait) > 1:
                    waits = list(si.on_wait)
                    for j, w in enumerate(waits[:-1]):
                        nop = mybir.InstNoOp(
                            name=f"{inst.name}_sw{j}", ins=[], outs=[],
                            engine=inst.engine,
                            sync_info=mybir.SyncInfo(on_wait=[w], on_update=[]),
                        )
                        out.append(nop)
                        n += 1
                    inst.sync_info = mybir.SyncInfo(
                        on_wait=[waits[-1]], on_update=list(si.on_update or []))
                out.append(inst)
            if len(out) != len(il):
                il[:] = out
    return n


def _ap(base, offset_elems, dims):
    """AP from a base tile AP with explicit free [step,count] dims (elements)."""
    return bass.AP(tensor=base.tensor, offset=base.offset + offset_elems,
                   ap=[list(base.ap[0])] + [list(d) for d in dims])


def _interp_w(h):
    """wy for rows 1..2h-2 of align_corners h->2h bilinear resize."""
    ys = np.linspace(0.0, np.float32(h - 1.0), 2 * h).astype(np.float64)
    wy = (ys - np.floor(ys)).astype(np.float32)
    return wy[1:2 * h - 1]


def host_prep(dw_w, dw_b, pw_w, pw_b, attn_w, attn_b, fc1_w, fc1_b, fc2_w, fc2_b):
    ext = {}
    dg = np.zeros((5, 128, 9 * 2 * 128), np.float32)
    dgv = dg.reshape(5, 128, 9, 2, 128)
    r = np.arange(128)
    for s, cv in enumerate(CONVS):
        for t, (dy, dx) in enumerate(TAPS):
            for cb in range(2):
                dgv[s, r, t, cb, r] = dw_w[cv, cb * 128 + r, 0, dy, dx]
    ext["dwdiag"] = dg.astype(ml_dtypes.bfloat16)
    b = np.zeros((128, 10), np.float32)
    pb = np.zeros((128, 10), np.float32)
    for s, cv in enumerate(CONVS):
        for cb in range(2):
            b[:, s * 2 + cb] = dw_b[cv, cb * 128:(cb + 1) * 128]
            pb[:, s * 2 + cb] = pw_b[cv, cb * 128:(cb + 1) * 128]
    ext["dwb"] = b
    ext["pwb"] = pb
    p = np.zeros((128, 5, 2, 2, 128), np.float32)
    for s, cv in enumerate(CONVS):
        w = pw_w[cv, :, :, 0, 0]
        for kcb in range(2):
            for ocb in range(2):
                p[:, s, kcb, ocb, :] = w[ocb * 128:(ocb + 1) * 128,
                                         kcb * 128:(kcb + 1) * 128].T
    ext["pwt"] = p.reshape(128, -1).astype(ml_dtypes.bfloat16)
    aw = attn_w.reshape(C).astype(np.float32)
    a = np.zeros((128, 6), np.float32)
    for l, (H, _) in enumerate(LEVELS):
        px = H * H
        a[:, l * 2 + 0] = aw[:128] / (px * 6.0)
        a[:, l * 2 + 1] = aw[128:] / (px * 6.0)
    ext["awt"] = a
    ext["attn_ab"] = np.full((1, 1), float(np.asarray(attn_b).reshape(-1)[0]) / 6.0,
                             np.float32)
    f1 = np.zeros((128, 3, 2, 64), np.float32)
    w1t = fc1_w.T.astype(np.float32)
    for l, (H, _) in enumerate(LEVELS):
        px = H * H
        f1[:, l, 0, :] = w1t[:128] / px
        f1[:, l, 1, :] = w1t[128:] / px
    ext["fc1t"] = f1.reshape(128, -1)
    ext["fc1b"] = fc1_b.reshape(64, 1).astype(np.float32)
    w2t = (fc2_w.T / 6.0).astype(np.float32)          # [64, 1024]
    order = [0, 128, 512, 640, 256, 384, 768, 896]    # a1,a1,a2,a2,b1,b1,b2,b2
    f2 = np.zeros((64, 8, 128), np.float32)
    f2b = np.zeros((128, 8), np.float32)
    for j, st in enumerate(order):
        f2[:, j, :] = w2t[:, st:st + 128]
        f2b[:, j] = fc2_b[st:st + 128] / 6.0 + 0.5
    ext["fc2t"] = f2.reshape(64, -1)
    ext["fc2b"] = f2b
    for nm, h in (("umap0", 32), ("umap1", 16)):
        wy = _interp_w(h).reshape(h - 1, 2)     # wy[j, q] for out row 1+2j+q
        n = (h - 1) * h + (h - 1)
        mm = np.zeros((1, 2, n), np.float32)
        for q in range(2):
            mm[0, q, 0:(h - 1) * h] = np.repeat(wy[:, q], h)
            mm[0, q, (h - 1) * h:n] = wy[:, q]
        ext[nm] = mm.astype(ml_dtypes.bfloat16)
    ext["cbundle"] = np.concatenate(
        [ext.pop("dwb"), ext.pop("pwb"), ext.pop("awt"), ext.pop("fc2b"),
         ext.pop("fc1t")], axis=1).astype(np.float32)
    return ext


def build_program(reps=1):
    nc = bass.Bass()

    xs = {nm: nc.dram_tensor(nm, [128, 2, H, H], F32, kind="ExternalInput")
          for nm, H in SRC_H.items()}
    dwdiag_d = nc.dram_tensor("dwdiag", [5, 128, 9 * 2 * 128], BF16, kind="ExternalInput")
    pwt_d = nc.dram_tensor("pwt", [128, 5 * 2 * 2 * 128], BF16, kind="ExternalInput")
    cb_d = nc.dram_tensor("cbundle", [128, 418], F32, kind="ExternalInput")
    ab_d = nc.dram_tensor("attn_ab", [1, 1], F32, kind="ExternalInput")
    fc1b_d = nc.dram_tensor("fc1b", [64, 1], F32, kind="ExternalInput")
    fc2t_d = nc.dram_tensor("fc2t", [64, 8 * 128], F32, kind="ExternalInput")
    um_d = {64: nc.dram_tensor("umap0", [1, 2, 31 * 32 + 31], BF16, kind="ExternalInput"),
            32: nc.dram_tensor("umap1", [1, 2, 15 * 16 + 15], BF16, kind="ExternalInput")}
    outs = [nc.dram_tensor(f"y{l}", [128, 2, H * H], BF16, kind="ExternalOutput")
            for l, (H, _) in enumerate(LEVELS)]

    with tile.TileContext(nc) as tc:
        with (
            tc.tile_pool(name="const", bufs=1) as cpool,
            tc.tile_pool(name="x12", bufs=1) as x12pool,
            tc.tile_pool(name="diag", bufs=2) as dgpool,
            tc.tile_pool(name="work", bufs=1) as wpool,
            tc.tile_pool(name="small", bufs=2) as spool,
            tc.tile_pool(name="dwps", bufs=3, space="PSUM") as dwps,
            tc.tile_pool(name="pwps", bufs=2, space="PSUM") as pwps,
            tc.tile_pool(name="smps", bufs=1, space="PSUM") as smps,
        ):
            # ---------- constants (weights via ACT-side HWDGE ring) ----------
            cbun = cpool.tile([128, 418], F32, tag="cbun")
            nc.scalar.dma_start(cbun[:], cb_d[:])
            dwb_sb = cbun[:, 0:10]
            pwb_sb = cbun[:, 10:20]
            awt_sb = cbun[:, 20:26]
            fc2b_sb = cbun[:, 26:34]
            fc1t_sb = cbun[:, 34:418].rearrange("p (a b c) -> p a b c", a=3, b=2)
            pwt_sb = cpool.tile([128, 5, 2, 2, 128], BF16, tag="pwt")
            nc.scalar.dma_start(pwt_sb[:], pwt_d[:].rearrange(
                "p (a b c d) -> p a b c d", a=5, b=2, c=2))
            ab_sb = cpool.tile([1, 1], F32, tag="ab")
            nc.scalar.dma_start(ab_sb[:], ab_d[:])
            fc1b_sb = cpool.tile([64, 1], F32, tag="fc1b")
            nc.scalar.dma_start(fc1b_sb[:], fc1b_d[:])
            fc2t_sb = cpool.tile([64, 8, 128], F32, tag="fc2t")
            nc.scalar.dma_start(fc2t_sb[:], fc2t_d[:].rearrange("p (a b) -> p a b", a=8))
            ones_sb = cpool.tile([1, 128], F32, tag="ones")
            nc.vector.memset(ones_sb[:], 1.0)
            hmapd_sb = {}
            for HH in (64, 32):
                h = HH // 2
                n = (h - 1) * h + (h - 1)   # dense h-map + w-map per phase
                m = cpool.tile([128, 2, n], BF16, tag=f"umap{HH}")
                nc.scalar.dma_start(m[:], um_d[HH][:].to_broadcast([128, 2, n]))
                hmaps = [m[:, q, 0:(h - 1) * h].rearrange(
                    "p (a b) -> p a b", a=h - 1) for q in range(2)]
                wmaps = [m[:, q, (h - 1) * h:n] for q in range(2)]
                hmapd_sb[HH] = (hmaps, wmaps)

            def load_pad(pool, stpool, nm):
                H = SRC_H[nm]
                P = H + 2
                xps = []
                for cb in range(2):
                    xp = pool.tile([128, P, P], BF16, tag=f"xpad_{nm}{cb}")
                    nc.gpsimd.memset(
                        _ap(xp[:], 0, [[(P - 1) * P, 2], [1, P]]), 0.0)
                    nc.gpsimd.memset(
                        _ap(xp[:], P, [[P, P - 2], [P - 1, 2]]), 0.0)
                    st = stpool.tile([128, H, H], F32, tag="xstage", bufs=1)
                    nc.sync.dma_start(st[:], xs[nm][:, cb])
                    nc.vector.tensor_copy(xp[:, 1:H + 1, 1:H + 1], st[:])
                    xps.append(xp)
                return xps

            res = {}
            acc = {}

            def conv(key, xp):
                cv, src, stride, Ho = W_ITEMS[key]
                s = CONV_SLOT[cv]
                Hp = SRC_H[src] + 2
                dwout = wpool.tile([128, 2, 64 * 64], BF16, tag="dwout")
                dg = dgpool.tile([128, 9, 2, 128], BF16, tag="dg")
                nc.scalar.dma_start(
                    dg[:], dwdiag_d[s].rearrange("p (t b f) -> p t b f", t=9, b=2))
                rows_per = max(1, 512 // Ho)
                ntiles = (Ho + rows_per - 1) // rows_per
                for cb in range(2):
                    xcb = xp[cb][:]
                    for ti in range(ntiles):
                        r0 = ti * rows_per
                        rows = min(rows_per, Ho - r0)
                        N = rows * Ho
                        ps = dwps.tile([128, 512], F32, tag="dw")
                        for t, (dy, dx) in enumerate(TAPS):
                            off = (dy + stride * r0) * Hp + dx
                            rhs = _ap(xcb, off,
                                      [[stride * Hp, rows], [stride, Ho]])
                            nc.tensor.matmul(ps[:, :N], dg[:, t, cb, :], rhs,
                                             start=(t == 0), stop=(t == 8))
                        nc.scalar.activation(
                            out=dwout[:, cb, r0 * Ho:(r0 + rows) * Ho],
                            in_=ps[:, :N], func=ACT.Silu,
                            bias=dwb_sb[:, s * 2 + cb:s * 2 + cb + 1])
                br = wpool.tile([128, 2, Ho, Ho], BF16, tag=f"br_{key}")
                ac = spool.tile([128, 2, 8], F32, tag=f"acc_{key}", bufs=1)
                nc.vector.memset(ac[:], 0.0)
                for ocb in range(2):
                    for ti in range(ntiles):
                        r0 = ti * rows_per
                        rows = min(rows_per, Ho - r0)
                        N = rows * Ho
                        ps = pwps.tile([128, 512], F32, tag="pw")
                        for kcb in range(2):
                            nc.tensor.matmul(
                                ps[:, :N], pwt_sb[:, s, kcb, ocb, :],
                                dwout[:, kcb, r0 * Ho:(r0 + rows) * Ho],
                                start=(kcb == 0), stop=(kcb == 1))
                        nc.scalar.activation(
                            out=br[:, ocb, r0:r0 + rows, :], in_=ps[:, :N],
                            func=ACT.Silu,
                            bias=pwb_sb[:, s * 2 + ocb:s * 2 + ocb + 1],
                            accum_out=ac[:, ocb, ti:ti + 1])
                res[key] = br
                acc[key] = ac

            def upsample(key, H):
                """res[key] [128,2,h,h] -> big [128,2,H,H]; overwrites acc[key]
                slots 0/1 with sums of the upsampled tensor."""
                h = H // 2
                src = res[key]
                ac = acc[key]
                big = wpool.tile([128, 2, H, H], BF16, tag=f"upbig{H}")
                hmap, wmap = hmapd_sb[H]
                for cb in range(2):
                    sc = src[:, cb]                       # [128, h, h]
                    d = wpool.tile([128, h - 1, h], BF16, tag=f"sC{H}")
                    nc.vector.tensor_tensor(out=d[:], in0=sc[:, 1:, :],
                                            in1=sc[:, :h - 1, :], op=ALU.subtract)
                    mid = wpool.tile([128, H, h], BF16, tag=f"sB{H}")
                    tb = wpool.tile([128, h - 1, h], BF16, tag=f"sA{H}")
                    for q in range(2):
                        # tb = d * wy_phase_q (all dense, 2x mode)
                        nc.vector.tensor_tensor(
                            out=tb[:], in0=d[:], in1=hmap[q], op=ALU.mult)
                        # mid rows (1+q), (3+q), ... = tb + src rows 0..h-2
                        nc.vector.tensor_tensor(
                            out=_ap(mid[:], (1 + q) * h, [[2 * h, h - 1], [1, h]]),
                            in0=tb[:],
                            in1=sc[:, 0:h - 1, :],
                            op=ALU.add)
                    nc.vector.tensor_copy(
                        _ap(mid[:], 0, [[(H - 1) * h, 2], [1, h]]),
                        _ap(sc, 0, [[(h - 1) * h, 2], [1, h]]))
                    d2 = wpool.tile([128, H, h - 1], BF16, tag=f"sC{H}")
                    nc.vector.tensor_tensor(out=d2[:], in0=mid[:, :, 1:],
                                            in1=mid[:, :, :h - 1], op=ALU.subtract)
                    t2 = wpool.tile([128, H, h - 1, 2], BF16, tag=f"sA{H}")
                    for q in range(2):
                        # t2 phase q = d2 * wx_phase_q (dense ins, strided out)
                        nc.vector.tensor_tensor(
                            out=_ap(t2[:], q, [[(h - 1) * 2, H], [2, h - 1]]),
                            in0=d2[:],
                            in1=_ap(wmap[q], 0, [[0, H], [1, h - 1]]),
                            op=ALU.mult)
                    for q in range(2):
                        nc.vector.scalar_tensor_tensor(
                            out=_ap(big[:], cb * H * H + 1 + q, [[H, H], [2, h - 1]]),
                            in0=_ap(t2[:], q, [[(h - 1) * 2, H], [2, h - 1]]),
                            scalar=1.0,
                            in1=mid[:, :, 0:h - 1],
                            op0=ALU.mult, op1=ALU.add,
                            accum_out=ac[:, cb, q:q + 1])
                    nc.vector.tensor_scalar(
                        out=_ap(big[:], cb * H * H, [[H, H], [H - 1, 2]]),
                        in0=_ap(mid[:], 0, [[h, H], [h - 1, 2]]),
                        scalar1=1.0, scalar2=0.0, op0=ALU.mult, op1=ALU.add,
                        accum_out=ac[:, cb, 2:3])
                return big

            def level_post(l, H):
                W = H
                branches = LEVELS[l][1]
                k = len(branches)
                lv = [(key, ups[key] if up else res[key]) for key, up in branches]
                # attention scalars
                aps_t = smps.tile([1, 8], F32, tag="aps")
                svec = spool.tile([128, 2, 8], F32, tag=f"svec{l}", bufs=1)
                for j, (key, _) in enumerate(lv):
                    nc.vector.tensor_reduce(
                        out=svec[:, :, j], in_=acc[key][:], axis=AX.X, op=ALU.add)
                    for cb in range(2):
                        nc.tensor.matmul(aps_t[:, j:j + 1], svec[:, cb, j:j + 1],
                                         awt_sb[:, l * 2 + cb:l * 2 + cb + 1],
                                         start=(cb == 0), stop=(cb == 1))
                a_sb = spool.tile([1, 8], F32, tag=f"asb{l}", bufs=1)
                nc.scalar.activation(out=a_sb[:, :k], in_=aps_t[:, :k],
                                     func=ACT.Relu, bias=ab_sb[:])
                nc.vector.tensor_scalar(out=a_sb[:, :k], in0=a_sb[:, :k],
                                        scalar1=0.5, scalar2=1.0,
                                        op0=ALU.add, op1=ALU.min)
                s_sb = spool.tile([128, 8], F32, tag=f"ssb{l}", bufs=1)
                bps = smps.tile([128, 8], F32, tag="f2ps")
                nc.tensor.matmul(bps[:, :k], ones_sb[:], a_sb[:, :k],
                                 start=True, stop=True)
                nc.vector.tensor_copy(s_sb[:, :k], bps[:, :k])

                # weighted branch max into padded buffer
                P2 = W + 2
                pad = wpool.tile([128, 2, H + 2, P2], BF16, tag=f"pad{H}")
                nc.gpsimd.memset(
                    _ap(pad[:], 0, [[P2 * (H + 2), 2], [(H + 1) * P2, 2], [1, P2]]),
                    NEG)
                nc.gpsimd.memset(
                    _ap(pad[:], P2, [[P2 * (H + 2), 2], [P2, H], [P2 - 1, 2]]), NEG)
                ybuf = wpool.tile([128, 2, H, W], BF16, tag=f"ybuf{H}")
                accy = spool.tile([128, 2], F32, tag=f"accy{l}", bufs=1)
                for cb in range(2):
                    wt = wpool.tile([128, H, W], BF16, tag=f"sA{H}")
                    nc.vector.tensor_scalar_mul(wt[:], lv[0][1][:, cb],
                                                s_sb[:, 0:1])
                    for j in range(1, k):
                        dst = pad[:, cb, 1:H + 1, 1:W + 1] if j == k - 1 else wt[:]
                        nc.vector.scalar_tensor_tensor(
                            out=dst, in0=lv[j][1][:, cb], scalar=s_sb[:, j:j + 1],
                            in1=wt[:], op0=ALU.mult, op1=ALU.max)
                    # 3x3 maxpool, separable
                    v1 = wpool.tile([128, H + 1, P2], BF16, tag=f"sB{H}")
                    nc.vector.tensor_tensor(out=v1[:], in0=pad[:, cb, 0:H + 1, :],
                                            in1=pad[:, cb, 1:H + 2, :], op=ALU.max)
                    v2 = wpool.tile([128, H, P2], BF16, tag=f"sA{H}")
                    nc.vector.tensor_tensor(out=v2[:], in0=v1[:, 0:H, :],
                                            in1=v1[:, 1:H + 1, :], op=ALU.max)
                    h1 = wpool.tile([128, H, W + 1], BF16, tag=f"sB{H}")
                    nc.vector.tensor_tensor(out=h1[:], in0=v2[:, :, 0:W + 1],
                                            in1=v2[:, :, 1:W + 2], op=ALU.max)
                    nc.vector.scalar_tensor_tensor(
                        out=ybuf[:, cb], in0=h1[:, :, 0:W], scalar=1.0,
                        in1=h1[:, :, 1:W + 1], op0=ALU.mult, op1=ALU.max,
                        accum_out=accy[:, cb:cb + 1])

                # DyReLU scalars
                f1ps = smps.tile([64, 1], F32, tag="f1ps")
                for cb in range(2):
                    nc.tensor.matmul(f1ps[:], fc1t_sb[:, l, cb, :],
                                     accy[:, cb:cb + 1],
                                     start=(cb == 0), stop=(cb == 1))
                y2 = spool.tile([64, 1], F32, tag=f"y2{l}", bufs=1)
                nc.scalar.activation(out=y2[:], in_=f1ps[:], func=ACT.Relu,
                                     bias=fc1b_sb[:])
                f2ps = smps.tile([128, 8], F32, tag="f2ps")
                for j in range(8):
                    nc.tensor.matmul(f2ps[:, j:j + 1], fc2t_sb[:, j, :], y2[:],
                                     start=True, stop=True)
                hs = spool.tile([128, 8], F32, tag=f"hs{l}", bufs=1)
                nc.vector.tensor_tensor(out=hs[:], in0=f2ps[:], in1=fc2b_sb[:],
                                        op=ALU.add)
                nc.vector.tensor_scalar(out=hs[:], in0=hs[:], scalar1=0.0,
                                        scalar2=1.0, op0=ALU.max, op1=ALU.min)
                scal = spool.tile([128, 8], F32, tag=f"scal{l}", bufs=1)
                nc.vector.tensor_scalar(out=scal[:, 0:2], in0=hs[:, 0:2],
                                        scalar1=2.0, scalar2=None, op0=ALU.mult)
                nc.vector.tensor_scalar(out=scal[:, 2:4], in0=hs[:, 2:4],
                                        scalar1=2.0, scalar2=-1.0,
                                        op0=ALU.mult, op1=ALU.add)
                nc.vector.tensor_scalar(out=scal[:, 4:8], in0=hs[:, 4:8],
                                        scalar1=-0.5, scalar2=None, op0=ALU.add)
                # DyReLU apply (writes back into ybuf), then store
                for cb in range(2):
                    yv = ybuf[:, cb].rearrange("p a b -> p (a b)")
                    t1 = wpool.tile([128, H * W], BF16, tag=f"sA{H}")
                    t2b = wpool.tile([128, H * W], BF16, tag=f"sB{H}")
                    nc.vector.tensor_scalar(out=t1[:], in0=yv,
                                            scalar1=scal[:, 0 + cb:1 + cb],
                                            scalar2=scal[:, 4 + cb:5 + cb],
                                            op0=ALU.mult, op1=ALU.add)
                    nc.vector.tensor_scalar(out=t2b[:], in0=yv,
                                            scalar1=scal[:, 2 + cb:3 + cb],
                                            scalar2=scal[:, 6 + cb:7 + cb],
                                            op0=ALU.mult, op1=ALU.add)
                    nc.vector.tensor_tensor(out=yv, in0=t1[:], in1=t2b[:],
                                            op=ALU.max)
                nc.sync.dma_start(outs[l][:],
                                  ybuf[:].rearrange("p b a c -> p b (a c)"))

            for rep in range(reps):
                ups = {}

                def body(xp0, xp1, xp2):
                    conv("c1x1", xp1)
                    ups["c1x1"] = upsample("c1x1", 64)
                    conv("c1x0", xp0)
                    conv("c5x0", xp0)
                    conv("c4x2", xp2)
                    ups["c4x2"] = upsample("c4x2", 32)

                if reps == 1:
                    with tc.tile_pool(name="x0p", bufs=1) as x0pool:
                        xp1 = load_pad(x12pool, x0pool, "x1")
                        xp0 = load_pad(x0pool, x0pool, "x0")
                        xp2 = load_pad(x12pool, x0pool, "x2")
                        body(xp0, xp1, xp2)
                else:
                    xp1 = load_pad(x12pool, x12pool, "x1")
                    xp0 = load_pad(x12pool, x12pool, "x0")
                    xp2 = load_pad(x12pool, x12pool, "x2")
                    body(xp0, xp1, xp2)
                conv("c4x1", xp1)
                level_post(0, 64)
                level_post(1, 32)
                conv("c7x2", xp2)
                conv("c8x1", xp1)
                level_post(2, 16)

    _split_multiwaits(nc)
    return nc


_CACHE = {}


def _get_program(reps=1):
    if reps not in _CACHE:
        _CACHE[reps] = build_program(reps)
    return _CACHE[reps]


def kernel(x0, x1, x2, dw_w, dw_b, pw_w, pw_b, attn_w, attn_b,
           fc1_w, fc1_b, fc2_w, fc2_b, reps=1):
    x0, x1, x2 = (np.asarray(a, np.float32) for a in (x0, x1, x2))
    ext = host_prep(np.asarray(dw_w), np.asarray(dw_b), np.asarray(pw_w),
                    np.asarray(pw_b), np.asarray(attn_w), np.asarray(attn_b),
                    np.asarray(fc1_w), np.asarray(fc1_b), np.asarray(fc2_w),
                    np.asarray(fc2_b))
    B = x0.shape[0]
    in_maps = []
    for c in range(B):
        m = dict(ext)
        for nm, arr in (("x0", x0), ("x1", x1), ("x2", x2)):
            H = arr.shape[2]
            m[nm] = np.ascontiguousarray(
                arr[c].reshape(2, 128, H, H).transpose(1, 0, 2, 3))
        in_maps.append(m)
    nc = _get_program(reps)
    r = run_bass_kernel_spmd(nc, in_maps, core_ids=list(range(N_CORES)))
    ys = []
    for l, (H, _) in enumerate(LEVELS):
        per = []
        for c in range(B):
            a = np.asarray(r.results[c][f"y{l}"])
            if a.dtype != ml_dtypes.bfloat16:
                a = a.view(ml_dtypes.bfloat16)
            a = a.astype(np.float32)
            a = a.reshape(128, 2, H, H).transpose(1, 0, 2, 3).reshape(C, H, H)
            per.append(a)
        ys.append(np.stack(per))
    return tuple(ys)
